# revision 31
# baseline (speedup 1.0000x reference)
import sys
from contextlib import ExitStack

import numpy as np

if "/opt/trn_rl_repo" not in sys.path:
    sys.path.insert(0, "/opt/trn_rl_repo")

from concourse import bacc, bass, tile
from concourse.bass_utils import run_bass_kernel_spmd

mybir = bass.mybir
F32 = mybir.dt.float32
BF16 = mybir.dt.bfloat16
AF = mybir.ActivationFunctionType

B, T, D, H = 16, 256, 128, 256
NC, NB = 8, 2
G3 = 3 * H

TRACE = False
LAST_EXEC_NS = None
_TIME_STATE = {}


def _build_program(loop_reps=1, skip_attn=False, skip_gru=False):
    nc = bacc.Bacc("TRN2", target_bir_lowering=False, debug=False, num_devices=NC)

    shapes = {
        "xT": ([128, T, NB], BF16),
        "xgT": ([128, T, NB], BF16),
        "wfeT": ([128, G3], BF16),
        "wfdT": ([128, G3], BF16),
        "whhTe": ([128, 2, G3], BF16),
        "whhTd": ([128, 2, G3], BF16),
        "biasE": ([128, 6], F32),
        "biasD": ([128, 6], F32),
        "wkT": ([128, 2, H], BF16),
        "wqT": ([128, 2, H], BF16),
        "vT": ([128, 2], BF16),
        "pT": ([128, NB, 2, T], F32),
    }
    dram_in = {
        name: nc.dram_tensor(name, shp, dt, kind="ExternalInput")
        for name, (shp, dt) in shapes.items()
    }
    out_dram = nc.dram_tensor("outT", [128, NB, 2, T], F32, kind="ExternalOutput")

    with tile.TileContext(nc) as tc, ExitStack() as ctx:
        if loop_reps > 1:
            ctx.enter_context(tc.For_i(0, loop_reps))
        wp = ctx.enter_context(tc.tile_pool(name="wp", bufs=1))
        sp = ctx.enter_context(tc.tile_pool(name="state", bufs=1))
        gp = ctx.enter_context(tc.tile_pool(name="gates", bufs=3))

        sb = {}
        for name, (shp, dt) in shapes.items():
            t_ = wp.tile(shp, dt, name=f"{name}_sb")
            nc.sync.dma_start(t_[:], dram_in[name][:])
            sb[name] = t_

        enc_h = sp.tile([128, T + 1, 2, NB], BF16, name="enc_h")
        dec_h = sp.tile([128, T + 1, 2, NB], BF16, name="dec_h")
        # gi slot order: 0=z, 1=r, 2=n (torch row order is r,z,n)
        gi_e = sp.tile([128, T, 3, 2, NB], F32, name="gi_e")
        gi_d = sp.tile([128, T, 3, 2, NB], F32, name="gi_d")

        # input projections: gi = x @ (Wih @ W_enc).T + bias, all t at once
        with tc.tile_pool(name="ps_gi", bufs=2, space="PSUM") as ps_gi:
            for gi, wf, bias, x in (
                (gi_e, sb["wfeT"], sb["biasE"], sb["xT"]),
                (gi_d, sb["wfdT"], sb["biasD"], sb["xgT"]),
            ):
                for gt in range(3):  # torch gate order r,z,n
                    slot = (1, 0, 2)[gt]
                    for o in range(2):
                        j = gt * 2 + o
                        ps = ps_gi.tile([128, T, NB], F32, name="gi_ps")
                        nc.tensor.matmul(
                            ps[:],
                            wf[:, j * 128 : (j + 1) * 128],
                            x[:],
                            start=True,
                            stop=True,
                        )
                        nc.scalar.activation(
                            gi[:, :, slot, o, :], ps[:], AF.Identity,
                            bias=bias[:, j : j + 1],
                        )

        nc.gpsimd.memset(enc_h[:, 0, :, :], 0.0)

        def gru_steps(h_all, gi, whhT):
            with tc.tile_pool(name="ps_gh", bufs=2, space="PSUM") as ps_gh:
                for t in range(T):
                    # psum slots 0=z, 1=r, 2=n; preload gi_z, gi_r and zero n,
                    # then accumulate with start=False (no bank zeroing)
                    gh = ps_gh.tile([128, 3, 2, NB], F32, name="gh")
                    nc.vector.tensor_copy(gh[:, 0:2, :, :], gi[:, t, 0:2, :, :])
                    nc.vector.memset(gh[:, 2, :, :], 0.0)
                    for g in range(3):      # PE order z, r, n
                        wg = (1, 0, 2)[g]   # torch weight row block
                        for o in range(2):
                            for kc in range(2):
                                gd = wg * H + o * 128
                                nc.tensor.matmul(
                                    gh[:, g, o, :],
                                    whhT[:, kc, gd : gd + 128],
                                    h_all[:, t, kc, :],
                                    start=False,
                                    stop=(kc == 1),
                                    skip_group_check=True,
                                )
                    zr = gp.tile([128, 2, 2, NB], F32, name="zr")
                    nc.scalar.activation(zr[:], gh[:, 0:2, :, :], AF.Sigmoid)
                    # off-critical-chain on gpsimd: zh = z*h, zc = 1-z
                    zh = gp.tile([128, 2, NB], F32, name="zh")
                    nc.gpsimd.tensor_mul(zh[:], zr[:, 0, :, :], h_all[:, t, :, :])
                    zc = gp.tile([128, 2, NB], F32, name="zc")
                    nc.gpsimd.tensor_scalar(
                        zc[:], zr[:, 0, :, :], -1.0, 1.0,
                        mybir.AluOpType.mult, mybir.AluOpType.add,
                    )
                    nm = gp.tile([128, 2, NB], F32, name="nm")
                    nc.vector.tensor_mul(nm[:], gh[:, 2, :, :], zr[:, 1, :, :])
                    ns = gp.tile([128, 2, NB], F32, name="ns")
                    nc.vector.tensor_add(ns[:], nm[:], gi[:, t, 2, :, :])
                    n = gp.tile([128, 2, NB], F32, name="n")
                    nc.scalar.activation(n[:], ns[:], AF.Tanh)
                    zcn = gp.tile([128, 2, NB], F32, name="zcn")
                    nc.vector.tensor_mul(zcn[:], n[:], zc[:])
                    nc.vector.tensor_add(h_all[:, t + 1, :, :], zcn[:], zh[:])

        if not skip_gru:
            gru_steps(enc_h, gi_e, sb["whhTe"])
            nc.vector.tensor_copy(dec_h[:, 0, :, :], enc_h[:, T, :, :])
            gru_steps(dec_h, gi_d, sb["whhTd"])
        else:
            nc.gpsimd.memset(enc_h[:], 0.125)
            nc.gpsimd.memset(dec_h[:], 0.125)

        def attn_block():
            # attention in permuted domain: scores_perm[t,j] =
            #   sum_a v[a]*tanh(q[t,a] + k[perm[j],a]); only j>=t needed
            # (host fills j<t with exactly -1e9 and unpermutes)
            q_sb = sp.tile([128, NB, 2, T], F32, name="q_sb")
            kt_sb = sp.tile([128, NB, 2, H], F32, name="kt_sb")
            kp_sb = sp.tile([128, NB, 2, T], F32, name="kp_sb")
            with tc.tile_pool(name="ps_at", bufs=2, space="PSUM") as ps_at:
                for bi in range(NB):
                    for ac in range(2):
                        ps = ps_at.tile([128, T], F32, name="proj_ps")
                        for kc in range(2):
                            nc.tensor.matmul(
                                ps[:],
                                sb["wqT"][:, kc, ac * 128 : (ac + 1) * 128],
                                dec_h[:, 1 : T + 1, kc, bi],
                                start=(kc == 0),
                                stop=(kc == 1),
                            )
                        nc.scalar.activation(q_sb[:, bi, ac, :], ps[:], AF.Identity, bias=0.0)
                # k with s on partitions: kt[s,a] = sum_h enc_h[h,s]*wkT[h,a]
                for bi in range(NB):
                    for sc in range(2):
                        ps = ps_at.tile([128, H], F32, name="kt_ps")
                        for kc in range(2):
                            nc.tensor.matmul(
                                ps[:],
                                enc_h[:, 1 + sc * 128 : 1 + (sc + 1) * 128, kc, bi],
                                sb["wkT"][:, kc, :],
                                start=(kc == 0),
                                stop=(kc == 1),
                            )
                        nc.vector.tensor_copy(kt_sb[:, bi, sc, :], ps[:])
                # permute: kp[a,j] = k[perm[j],a] via one-hot matmul (exact)
                for bi in range(NB):
                    for ach in range(2):
                        ps = ps_at.tile([128, T], F32, name="kp_ps")
                        for sc in range(2):
                            nc.tensor.matmul(
                                ps[:],
                                kt_sb[:, bi, sc, ach * 128 : (ach + 1) * 128],
                                sb["pT"][:, bi, sc, :],
                                start=(sc == 0),
                                stop=(sc == 1),
                            )
                        nc.vector.tensor_copy(kp_sb[:, bi, ach, :], ps[:])

            TB = 8
            with (
                tc.tile_pool(name="ps_sc", bufs=4, space="PSUM") as ps_sc,
                tc.tile_pool(name="attn", bufs=2) as ap_,
            ):
                nadd = 0
                for bi in range(NB):
                    for tb in range(0, T, TB):
                        Lj = T - tb
                        # score rows are computed SS at a time into one psum
                        # row [1, SS*Lj] (matmul out base partition must be 0)
                        SS = 1
                        while SS * 2 <= TB and SS * 2 * Lj <= 512:
                            SS *= 2
                        kq = ap_.tile([128, 2, TB, Lj], F32, name="kq", tag="kq")
                        for ti in range(TB):
                            for ac in range(2):
                                # vector also drains psum rows; weight it 1:3
                                eng = nc.vector if (nadd % 4 == 0) else nc.gpsimd
                                nadd += 1
                                eng.tensor_scalar_add(
                                    kq[:, ac, ti, :],
                                    kp_sb[:, bi, ac, tb:T],
                                    q_sb[:, bi, ac, tb + ti : tb + ti + 1],
                                )
                        th = ap_.tile([128, 2, TB, Lj], BF16, name="th", tag="th")
                        nc.scalar.activation(th[:], kq[:], AF.Tanh)
                        for ss in range(0, TB, SS):
                            t0 = tb + ss
                            hh, pr = divmod(t0, 128)
                            ps = ps_sc.tile([128, 512], F32, name="srow_ps")
                            for ac in range(2):
                                nc.tensor.matmul(
                                    ps[0:1, 0 : SS * Lj],
                                    sb["vT"][:, ac : ac + 1],
                                    th[:, ac, ss : ss + SS, :],
                                    start=(ac == 0),
                                    stop=(ac == 1),
                                )
                            srow = ap_.tile([1, 512], F32, name="srow", tag="srow", bufs=4)
                            nc.vector.tensor_copy(srow[0:1, 0 : SS * Lj], ps[0:1, 0 : SS * Lj])
                            nc.sync.dma_start(
                                out_dram[pr : pr + SS, bi, hh, tb : tb + Lj],
                                srow[0:1, 0 : SS * Lj],
                            )

        if skip_attn:
            for bi in range(NB):
                for s in range(2):
                    nc.sync.dma_start(out_dram[:, bi, s, :], gi_e[:, :, 0, 0, 0])
        else:
            attn_block()

    if not nc.is_finalized():
        nc.finalize()
    return nc, list(shapes.keys())


def kernel(**inputs):
    global LAST_EXEC_NS
    from ml_dtypes import bfloat16

    x = np.ascontiguousarray(np.asarray(inputs["inputs"], dtype=np.float32))
    targets = np.asarray(inputs["targets"]).astype(np.int64)
    f64 = np.float64

    def fuse(Wih, bih, bhh, W_enc, b_enc):
        Wf = (Wih.astype(f64) @ W_enc.astype(f64)).astype(np.float32)
        bf = (
            Wih.astype(f64) @ b_enc.astype(f64)
            + bih.astype(f64)
            + bhh.astype(f64)
        ).astype(np.float32)
        return Wf, bf

    W_enc = np.asarray(inputs["W_enc"], dtype=np.float32)
    b_enc = np.asarray(inputs["b_enc"], dtype=np.float32)
    Wfe, bfe = fuse(
        np.asarray(inputs["enc_Wih"], dtype=np.float32),
        np.asarray(inputs["enc_bih"], dtype=np.float32),
        np.asarray(inputs["enc_bhh"], dtype=np.float32),
        W_enc, b_enc,
    )
    Wfd, bfd = fuse(
        np.asarray(inputs["dec_Wih"], dtype=np.float32),
        np.asarray(inputs["dec_bih"], dtype=np.float32),
        np.asarray(inputs["dec_bhh"], dtype=np.float32),
        W_enc, b_enc,
    )

    def whhT_layout(Whh):
        return np.ascontiguousarray(
            np.asarray(Whh, dtype=np.float32).T.reshape(2, 128, G3).transpose(1, 0, 2)
        )

    def hT_layout(Wm):  # [H, H] -> [128, 2, H]
        return np.ascontiguousarray(
            np.asarray(Wm, dtype=np.float32).T.reshape(2, 128, H).transpose(1, 0, 2)
        )

    whhTe = whhT_layout(inputs["enc_Whh"])
    whhTd = whhT_layout(inputs["dec_Whh"])
    wkT = hT_layout(inputs["Wk"])
    wqT = hT_layout(inputs["Wq"])
    vT = np.ascontiguousarray(
        np.asarray(inputs["v"], dtype=np.float32).reshape(2, 128).T
    )
    wfeT = np.ascontiguousarray(Wfe.T)
    wfdT = np.ascontiguousarray(Wfd.T)
    biasE = np.ascontiguousarray(bfe.reshape(6, 128).T)
    biasD = np.ascontiguousarray(bfd.reshape(6, 128).T)

    dec_idx = np.roll(targets, 1, axis=1)
    xg = np.take_along_axis(x, dec_idx[:, :, None], axis=1)

    # one-hot of the permutation: pT[p, bi, sc, j] = 1 iff targets[b,j]==sc*128+p
    onehot = (targets[:, :, None] == np.arange(T)[None, None, :]).astype(np.float32)

    nc, in_names = _build_program()

    in_maps = []
    for c in range(NC):
        bs = slice(c * NB, (c + 1) * NB)
        xc = np.ascontiguousarray(x[bs].transpose(2, 1, 0))          # [128, T, NB]
        xgc = np.ascontiguousarray(xg[bs].transpose(2, 1, 0))
        pc = np.ascontiguousarray(
            onehot[bs].transpose(0, 2, 1).reshape(NB, 2, 128, T).transpose(2, 0, 1, 3)
        )                                                            # [128, NB, 2, T]
        in_maps.append({
            "xT": xc.astype(bfloat16), "xgT": xgc.astype(bfloat16),
            "wfeT": wfeT.astype(bfloat16), "wfdT": wfdT.astype(bfloat16),
            "whhTe": whhTe.astype(bfloat16), "whhTd": whhTd.astype(bfloat16),
            "biasE": biasE, "biasD": biasD,
            "wkT": wkT.astype(bfloat16), "wqT": wqT.astype(bfloat16),
            "vT": vT.astype(bfloat16),
            "pT": pc,
        })

    br = run_bass_kernel_spmd(nc, in_maps, list(range(NC)), trace=TRACE)
    if TRACE:
        LAST_EXEC_NS = br.exec_time_ns
    _TIME_STATE["nc"] = nc
    _TIME_STATE["in_maps"] = in_maps

    logits = np.empty((B, T, T), dtype=np.float32)
    tri = np.arange(T)[None, :] >= np.arange(T)[:, None]
    for c in range(NC):
        outT = br.results[c]["outT"]                                 # [128, NB, 2, T]
        for bi in range(NB):
            b = c * NB + bi
            dev = outT[:, bi, :, :].transpose(1, 0, 2).reshape(T, T)  # [t, j]
            mapped = np.where(tri, dev, np.float32(-1e9))
            inv = np.empty(T, dtype=np.int64)
            inv[targets[b]] = np.arange(T)
            logits[b] = mapped[:, inv]
    return logits


def _make_pjrt_fn(nc, in_maps):
    import jax
    from concourse import bass2jax

    bass2jax.install_neuronx_cc_hook()
    partition_name = nc.partition_id_tensor.name if nc.partition_id_tensor else None
    in_names, out_names, out_avals, zero_outs = [], [], [], []
    for alloc in nc.m.functions[0].allocations:
        if not isinstance(alloc, bass.mybir.MemoryLocationSet):
            continue
        name = alloc.memorylocations[0].name
        if alloc.kind == "ExternalInput":
            if name != partition_name:
                in_names.append(name)
        elif alloc.kind == "ExternalOutput":
            shape = tuple(alloc.tensor_shape)
            dtype = bass.mybir.dt.np(alloc.dtype)
            out_avals.append(jax.core.ShapedArray(shape, dtype))
            out_names.append(name)
            zero_outs.append(np.zeros(shape, dtype))
    n_params = len(in_names)
    all_in_names = in_names + out_names
    if partition_name is not None:
        all_in_names = all_in_names + [partition_name]

    def _body(*args):
        operands = list(args)
        if partition_name is not None:
            operands.append(bass2jax.partition_id_tensor())
        return tuple(
            bass2jax._bass_exec_p.bind(
                *operands,
                out_avals=tuple(out_avals),
                in_names=tuple(all_in_names),
                out_names=tuple(out_names),
                lowering_input_output_aliases=(),
                sim_require_finite=True,
                sim_require_nnan=True,
                nc=nc,
            )
        )

    n_cores = len(in_maps)
    devices = jax.devices()[:n_cores]
    mesh = bass2jax.Mesh(np.asarray(devices), ("core",))
    P = bass2jax.PartitionSpec
    f = jax.jit(
        bass2jax.shard_map(
            _body,
            mesh=mesh,
            in_specs=(P("core"),) * (n_params + len(out_names)),
            out_specs=(P("core"),) * len(out_names),
            check_rep=False,
        ),
        keep_unused=True,
    )
    sharding = jax.sharding.NamedSharding(mesh, P("core"))
    dev_args = []
    for i, name in enumerate(in_names):
        g = np.concatenate([np.asarray(m[name]) for m in in_maps], axis=0)
        dev_args.append(jax.device_put(g, sharding))
    for z in zero_outs:
        g = np.concatenate([z] * n_cores, axis=0)
        dev_args.append(jax.device_put(g, sharding))
    return f, dev_args


def _time_fn(f, dev_args, reps):
    import time as _time

    import jax

    out = f(*dev_args)
    jax.block_until_ready(out)
    ts = []
    for _ in range(reps):
        t0 = _time.perf_counter_ns()
        out = f(*dev_args)
        jax.block_until_ready(out)
        ts.append(_time.perf_counter_ns() - t0)
    ts.sort()
    return ts[len(ts) // 2], ts


def _build_null_program():
    nc = bacc.Bacc("TRN2", target_bir_lowering=False, debug=False, num_devices=NC)
    din = nc.dram_tensor("nullin", [128, 1], F32, kind="ExternalInput")
    dout = nc.dram_tensor("nullout", [128, 1], F32, kind="ExternalOutput")
    with tile.TileContext(nc) as tc, tc.tile_pool(name="np_", bufs=1) as p:
        t_ = p.tile([128, 1], F32, name="t_")
        nc.sync.dma_start(t_[:], din[:])
        nc.sync.dma_start(dout[:], t_[:])
    if not nc.is_finalized():
        nc.finalize()
    return nc


def measure_exec_ns(reps=12, loop_reps=64):
    global LAST_EXEC_NS
    in_maps = _TIME_STATE["in_maps"]
    nc_r, _ = _build_program(loop_reps=loop_reps)
    f_r, dev_r = _make_pjrt_fn(nc_r, in_maps)
    t_r, ts_r = _time_fn(f_r, dev_r, reps)
    nc0 = _build_null_program()
    null_maps = [{"nullin": np.zeros((128, 1), np.float32)} for _ in range(NC)]
    f0, dev_args0 = _make_pjrt_fn(nc0, null_maps)
    t_null, ts_null = _time_fn(f0, dev_args0, reps)
    LAST_EXEC_NS = int(max(t_r - t_null, 0) / loop_reps)
    return {
        "exec_ns": LAST_EXEC_NS,
        "loop_reps": loop_reps,
        "amp_median_ns": t_r,
        "null_median_ns": t_null,
        "amp_min_ns": ts_r[0],
        "null_min_ns": ts_null[0],
        "exec_ns_min_based": int(max(ts_r[0] - ts_null[0], 0) / loop_reps),
    }
